# revision 56
# baseline (speedup 1.0000x reference)
"""Trainium2 Bass kernel for nn_Estor_concat (scatter_memory).

Math (exact reformulation of the reference):
  v_tag = (tag_emb @ Wv.T + bv) @ out_proj_w.T + out_proj_b          [T, H]
  W_eff[t, j] = sum_h v_tag[t, h] * ff1_w[j, t*H + h]               [T, H]
  counts[t, s] = #spans(tag=t, batch=b) covering s   (PE matmul over spans)
  h1 = relu(counts.T @ W_eff + ff1_b); h2 = h1 @ ff2.T + ff2_b
  x = [word_emb_b | h2]; LayerNorm+output folded:
  out = (raw - mu*c1) * rstd + c2,  raw = x @ (g*lin_w.T)

Sharding: data-parallel over batch (8 cores, 1 batch each); W_eff is
sharded over tags (2/core, ff1 rows sharded likewise) and combined with
one AllGather.  v_tag and c1/c2/g·lin_w are parameter-only constants,
folded on the host.  The device schedule dispatches the AllGather as
early as possible (gated only by the fp8 ff1 shard load + ~150ns of N=1
matmuls in transposed j-on-partitions form) and runs counts + the
word-embedding part of the output accumulation inside the collective's
~15.6us latency window, with a dummy-matmul chain keeping the PE pstate
warm.  h2 runs as fp8 DoubleRow matmuls (half row-cost); all LayerNorm
stats are computed in s-on-partitions layout ([128, n] tiles) so the
tail is ~40-160ns ops instead of [1, 512] single-partition ops.
"""

import ml_dtypes
import numpy as np

import concourse.bacc as bacc
import concourse.bass as bass
import concourse.mybir as mybir
import concourse.tile as tile
from concourse.bass_utils import run_bass_kernel_spmd

T, B, S, H = 16, 8, 512, 768
H2 = 384
NEW_H = H + H2          # 1152
NL = 33                 # num labels
EPS = 1e-12
NCORES = 8
TPC = T // NCORES       # tags per core = 2
KC_H = H // 128         # 6
KC_H2 = H2 // 128       # 3
KC_F = NEW_H // 128     # 9
P = 128
NSC = S // 128          # 4 s-chunks
STW = NL + 2            # stats cols per s-chunk: 33 raw + sum + sumsq

F32 = mybir.dt.float32
F16 = mybir.dt.float16
F8 = mybir.dt.float8e4

# aux (fp16) column offsets
A_IS = 0                # iota_s [512]
A_IT = 512              # iota_t [16]
A_VT = 528              # v_tag.T cols (kk*2 + tl) [12]


def build_kernel(nt: int, DUMMY_N: int = 61):
    nc = bacc.Bacc(
        "TRN2",
        target_bir_lowering=False,
        debug=False,
        enable_asserts=True,
        num_devices=NCORES,
    )

    def inp(name, shape, dtype=F32):
        return nc.dram_tensor(name, list(shape), dtype, kind="ExternalInput").ap()

    a_sps = 75
    a_spe = a_sps + nt
    a_spt = a_spe + nt

    aux = inp("aux", (P, A_VT + 12), F16)
    ff1blk = inp("ff1blk", (P, KC_H, TPC * KC_H * P), F8)
    we_t = inp("we_t", (P, KC_H, S), F16)
    ff2t = inp("ff2t", (P, KC_H, H2), F8)
    lwg2 = inp("lwg2", (P, KC_F, NL + 1), F16)
    lwcol = inp("lwcol", (P, 75 + 3 * nt), F32)

    out = nc.dram_tensor("out", [S, NL], F32, kind="ExternalOutput").ap()

    with tile.TileContext(nc) as tc:
        with (
            tc.tile_pool(name="singles", bufs=1) as singles,
            tc.tile_pool(name="spans", bufs=3) as spans,
            tc.tile_pool(name="work", bufs=3) as work,
            tc.tile_pool(name="work2", bufs=2) as work2,
            tc.tile_pool(name="stats", bufs=1) as stats,
            tc.tile_pool(name="fin", bufs=4) as fin,
            tc.tile_pool(name="ps_big", bufs=3, space="PSUM") as ps_big,
            tc.tile_pool(name="ps_h2", bufs=3, space="PSUM") as ps_h2,
            tc.tile_pool(name="ps_acc", bufs=1, space="PSUM") as ps_acc,
            tc.tile_pool(name="dram", bufs=1, space="DRAM") as dram,
        ):
            # ---- constants ------------------------------------------------
            ones_col = singles.tile([P, 1], F16)
            nc.vector.memset(ones_col, 1.0)
            eps_t = singles.tile([1, 1], F32)
            nc.vector.memset(eps_t, EPS)
            eps_col = singles.tile([P, 1], F32)
            nc.vector.memset(eps_col, EPS)
            scratch = singles.tile([1, 1], F32)

            # ---- DMA wave 1: the AllGather path (ff1 shard) gates all -----
            aux_sb = singles.tile([P, A_VT + 12], F16)
            nc.sync.dma_start(out=aux_sb, in_=aux)
            ff1_sb = singles.tile([P, KC_H, TPC * KC_H * P], F8)
            nc.sync.dma_start(out=ff1_sb[:, 0, :], in_=ff1blk[:, 0, :])
            nc.sync.dma_start(out=ff1_sb[:, 1, :], in_=ff1blk[:, 1, :])
            nc.scalar.dma_start(out=ff1_sb[:, 2, :], in_=ff1blk[:, 2, :])
            nc.gpsimd.dma_start(out=ff1_sb[:, 3, :], in_=ff1blk[:, 3, :])
            nc.gpsimd.dma_start(out=ff1_sb[:, 4, :], in_=ff1blk[:, 4, :])
            nc.gpsimd.dma_start(out=ff1_sb[:, 5, :], in_=ff1blk[:, 5, :])
            # ---- wave 2: everything needed during/after the collective ----
            we_sb = singles.tile([P, KC_H, S], F16)
            nc.sync.dma_start(out=we_sb[:, 0:3, :], in_=we_t[:, 0:3, :])
            nc.scalar.dma_start(out=we_sb[:, 3:6, :], in_=we_t[:, 3:6, :])
            ff2_sb = singles.tile([P, KC_H, H2], F8)
            nc.sync.dma_start(out=ff2_sb[:, 0:3, :], in_=ff2t[:, 0:3, :])
            nc.scalar.dma_start(out=ff2_sb[:, 3:6, :], in_=ff2t[:, 3:6, :])
            lwg2_sb = singles.tile([P, KC_F, NL + 1], F16)
            nc.scalar.dma_start(out=lwg2_sb, in_=lwg2)
            lwcol_sb = singles.tile([P, 75 + 3 * nt], F32)
            nc.scalar.dma_start(out=lwcol_sb, in_=lwcol)
            ff1b = lwcol_sb[:, 0:6]
            ff2b = lwcol_sb[:, 6:9]
            c1b = lwcol_sb[:, 9:42]
            c2b = lwcol_sb[:, 42:75]

            # prewarm act table set (relu+sqrt+square live in one set)
            nc.scalar.activation(
                out=scratch, in_=eps_t,
                func=mybir.ActivationFunctionType.Sqrt, bias=eps_t, scale=1.0,
            )

            # ---- W_eff^T: 72 N=1 matmuls, j on partitions -----------------
            # wT cols are tl-major (tl*6 + jc) so the transposed DRAM write
            # below lands as row-major [TPC, H].
            wT_sb = singles.tile([P, TPC * KC_H], F16)
            ps_stats = ps_acc.tile([P, NSC * STW + TPC * KC_H], F32, tag="stats")
            nc.vector.memset(ps_stats, 0.0)
            ps_wall = ps_stats[:, NSC * STW:]
            for jc in (0, 3, 1, 4, 5, 2):
                for tl in range(TPC):
                    col = tl * KC_H + jc
                    for kk in range(KC_H):
                        blk = tl * KC_H + kk
                        nc.tensor.matmul(
                            ps_wall[:, col:col + 1],
                            ff1_sb[:, jc, blk * P:(blk + 1) * P],
                            aux_sb[:, A_VT + kk * TPC + tl:A_VT + kk * TPC + tl + 1],
                            start=False, stop=False, skip_group_check=True,
                        )
            nc.vector.tensor_copy(out=wT_sb, in_=ps_wall)

            # ---- AllGather: [TPC, H] shard -> [T, H] ----------------------
            ag_in = dram.tile([TPC * KC_H, P], F16)
            nc.gpsimd.dma_start(out=ag_in.rearrange("a b -> b a"), in_=wT_sb)
            ag_out = dram.tile([T, H], F16)
            nc.gpsimd.collective_compute(
                "AllGather",
                mybir.AluOpType.bypass,
                replica_groups=[list(range(NCORES))],
                ins=[ag_in.opt()],
                outs=[ag_out.opt()],
            )
            weff_sb = singles.tile([T, H], F16)
            nc.sync.dma_start(out=weff_sb[:, 0:H // 2], in_=ag_out[:, 0:H // 2])
            nc.scalar.dma_start(out=weff_sb[:, H // 2:], in_=ag_out[:, H // 2:])

            # ================ overlapped with the AllGather ================
            # ---- counts: masks on DVE, accumulate on PE -------------------
            counts_ps = ps_acc.tile([T, S], F32, tag="counts")
            tc.tile_set_cur_wait(0.0042)
            for i in range(nt):
                lt_e = spans.tile([P, S], F16, tag="lt_e")
                lt_s = spans.tile([P, S], F16, tag="lt_s")
                mask = spans.tile([P, S], F16, tag="mask")
                nc.vector.tensor_scalar(
                    out=lt_e, in0=aux_sb[:, A_IS:A_IS + S],
                    scalar1=lwcol_sb[:, a_spe + i:a_spe + i + 1], scalar2=None,
                    op0=mybir.AluOpType.is_lt,
                )
                nc.vector.tensor_scalar(
                    out=lt_s, in0=aux_sb[:, A_IS:A_IS + S],
                    scalar1=lwcol_sb[:, a_sps + i:a_sps + i + 1], scalar2=None,
                    op0=mybir.AluOpType.is_ge,
                )
                nc.vector.tensor_mul(out=mask, in0=lt_e, in1=lt_s)
                onehot = spans.tile([P, T], F16, tag="onehot")
                nc.vector.tensor_scalar(
                    out=onehot, in0=aux_sb[:, A_IT:A_IT + T],
                    scalar1=lwcol_sb[:, a_spt + i:a_spt + i + 1], scalar2=None,
                    op0=mybir.AluOpType.is_equal,
                )
                nc.tensor.matmul(
                    counts_ps, onehot, mask,
                    start=(i == 0), stop=(i == nt - 1),
                )
            counts_sb = singles.tile([T, S], F16)
            nc.vector.tensor_copy(out=counts_sb, in_=counts_ps)
            tc.cur_wait_ts = None

            # ---- word-embedding part of raw/sum/sumsq, s-on-partitions ----
            # ps_stats cols per sc: [0:33]=raw, 33=sum, 34=sumsq
            for fc in range(KC_H):
                sq = work.tile([P, S], F16, tag="sq")
                nc.vector.tensor_mul(
                    out=sq, in0=we_sb[:, fc, :], in1=we_sb[:, fc, :]
                )
                for sc in range(NSC):
                    o = sc * STW
                    nc.tensor.matmul(
                        ps_stats[:, o:o + NL + 1],
                        we_sb[:, fc, sc * P:(sc + 1) * P],
                        lwg2_sb[:, fc, :],
                        start=False, stop=False, skip_group_check=True,
                    )
                    nc.tensor.matmul(
                        ps_stats[:, o + NL + 1:o + NL + 2],
                        sq[:, sc * P:(sc + 1) * P],
                        ones_col,
                        start=False, stop=False, skip_group_check=True,
                    )

            # ---- PE keep-warm chain through the collective window ---------
            # WAW-serialized dummy matmuls hold the PE pstate at full clock
            # until W_eff arrives; DUMMY_N is tuned so the chain ends exactly
            # at the weff load (overrun delays h1, undershoot resets pstate).
            for _ in range(DUMMY_N):
                ps_dum = ps_big.tile([TPC * KC_H, S], F32, tag="big")
                nc.tensor.matmul(
                    ps_dum, wT_sb, we_sb[:, 0, :], start=True, stop=True,
                )

            # ================ post-AllGather tail ==========================
            # h1 = relu(counts.T @ W_eff + ff1_b), stored transposed [H, S]
            h1r_sb = singles.tile([P, KC_H, S], F8)
            for kj in range(KC_H):
                # alternate pools so all six chunks get independent buffers
                pool = ps_big if kj % 2 == 0 else ps_h2
                ps = pool.tile([P, S], F32, tag="big" if kj % 2 == 0 else "h2")
                nc.tensor.matmul(
                    ps, weff_sb[:, kj * P:(kj + 1) * P], counts_sb,
                    start=True, stop=True,
                )
                if kj % 2 == 1:
                    nc.scalar.activation(
                        out=h1r_sb[:, kj, :], in_=ps,
                        func=mybir.ActivationFunctionType.Relu,
                        bias=ff1b[:, kj:kj + 1], scale=1.0,
                    )
                else:
                    nc.vector.tensor_scalar(
                        out=h1r_sb[:, kj, :], in0=ps,
                        scalar1=ff1b[:, kj:kj + 1], scalar2=0.0,
                        op0=mybir.AluOpType.add, op1=mybir.AluOpType.max,
                    )

            # h2 = relu_h1 @ ff2.T + ff2_b (fp8 DoubleRow), transposed [H2, S]
            xh2_sb = singles.tile([P, KC_H2, S], F16)
            for mc in range(KC_H2):
                ps = ps_h2.tile([P, S], F32, tag="h2")
                for kjp in range(KC_H // 2):
                    nc.tensor.matmul(
                        ps,
                        ff2_sb[:, 2 * kjp:2 * kjp + 2, mc * P:(mc + 1) * P],
                        h1r_sb[:, 2 * kjp:2 * kjp + 2, :],
                        start=(kjp == 0), stop=(kjp == KC_H // 2 - 1),
                        perf_mode=mybir.MatmulPerfMode.DoubleRow,
                    )
                if mc == 2:
                    # split the critical last chunk across both PSUM engines
                    h = S // 2
                    nc.scalar.activation(
                        out=xh2_sb[:, mc, 0:h], in_=ps[:, 0:h],
                        func=mybir.ActivationFunctionType.Identity,
                        bias=ff2b[:, mc:mc + 1], scale=1.0,
                    )
                    nc.vector.tensor_scalar(
                        out=xh2_sb[:, mc, h:], in0=ps[:, h:],
                        scalar1=ff2b[:, mc:mc + 1], scalar2=None,
                        op0=mybir.AluOpType.add,
                    )
                elif mc == 0:
                    nc.scalar.activation(
                        out=xh2_sb[:, mc, :], in_=ps,
                        func=mybir.ActivationFunctionType.Identity,
                        bias=ff2b[:, mc:mc + 1], scale=1.0,
                    )
                else:
                    nc.vector.tensor_scalar(
                        out=xh2_sb[:, mc, :], in0=ps,
                        scalar1=ff2b[:, mc:mc + 1], scalar2=None,
                        op0=mybir.AluOpType.add,
                    )
                sq = work2.tile([P, S], F16, tag="sqh")
                if mc == 2:
                    h = S // 2
                    nc.vector.tensor_mul(
                        out=sq[:, 0:h], in0=xh2_sb[:, mc, 0:h],
                        in1=xh2_sb[:, mc, 0:h],
                    )
                    nc.gpsimd.tensor_mul(
                        out=sq[:, h:], in0=xh2_sb[:, mc, h:],
                        in1=xh2_sb[:, mc, h:],
                    )
                else:
                    engq = nc.gpsimd if mc == 0 else nc.vector
                    engq.tensor_mul(
                        out=sq, in0=xh2_sb[:, mc, :], in1=xh2_sb[:, mc, :]
                    )
                for sc in range(NSC):
                    o = sc * STW
                    nc.tensor.matmul(
                        ps_stats[:, o:o + NL + 1],
                        xh2_sb[:, mc, sc * P:(sc + 1) * P],
                        lwg2_sb[:, KC_H + mc, :],
                        start=False, stop=False, skip_group_check=True,
                    )
                    nc.tensor.matmul(
                        ps_stats[:, o + NL + 1:o + NL + 2],
                        sq[:, sc * P:(sc + 1) * P],
                        ones_col,
                        start=False, stop=False, skip_group_check=True,
                    )

            # ---- stats + final normalize per s-chunk ----------------------
            mu_t = stats.tile([P, NSC], F32)
            nv_t = stats.tile([P, NSC], F32)
            sd_t = stats.tile([P, NSC], F32)
            rstd_t = stats.tile([P, NSC], F32)
            for sc in range(NSC):
                o = sc * STW
                nc.vector.tensor_scalar_mul(
                    out=mu_t[:, sc:sc + 1], in0=ps_stats[:, o + NL:o + NL + 1],
                    scalar1=1.0 / NEW_H,
                )
                # nv = N*mu^2 - sumsq  ->  var = -nv/N
                nc.vector.tensor_scalar(
                    out=nv_t[:, sc:sc + 1], in0=mu_t[:, sc:sc + 1],
                    scalar1=ps_stats[:, o + NL:o + NL + 1],
                    scalar2=ps_stats[:, o + NL + 1:o + NL + 2],
                    op0=mybir.AluOpType.mult, op1=mybir.AluOpType.subtract,
                )
                nc.scalar.activation(
                    out=sd_t[:, sc:sc + 1], in_=nv_t[:, sc:sc + 1],
                    func=mybir.ActivationFunctionType.Sqrt,
                    bias=eps_col, scale=-1.0 / NEW_H,
                )
                nc.vector.reciprocal(
                    out=rstd_t[:, sc:sc + 1], in_=sd_t[:, sc:sc + 1]
                )
                tmp = fin.tile([P, NL], F32, tag="tmp")
                nc.scalar.activation(
                    out=tmp, in_=c1b,
                    func=mybir.ActivationFunctionType.Identity,
                    bias=0.0, scale=mu_t[:, sc:sc + 1],
                )
                x1 = fin.tile([P, NL], F32, tag="x1")
                nc.vector.tensor_sub(
                    out=x1, in0=ps_stats[:, o:o + NL], in1=tmp
                )
                tt = fin.tile([P, NL], F32, tag="tt")
                nc.gpsimd.tensor_scalar_mul(
                    out=tt, in0=x1, scalar1=rstd_t[:, sc:sc + 1],
                )
                fo = fin.tile([P, NL], F32, tag="fo")
                nc.gpsimd.tensor_add(out=fo, in0=tt, in1=c2b)
                eng = (nc.sync, nc.scalar, nc.sync, nc.scalar)[sc]
                eng.dma_start(out=out[sc * P:(sc + 1) * P, :], in_=fo)

    nc.compile()
    return nc


def _chunked(a, kc):
    """[kc*128, N...] -> [128, kc, N...] (partition-major chunk layout)."""
    return np.ascontiguousarray(
        a.reshape(kc, P, *a.shape[1:]).transpose(1, 0, *range(2, a.ndim + 1))
    )


_CACHE = {}


def kernel(**inputs) -> np.ndarray:
    f16 = np.float16
    we = np.asarray(inputs["word_embedding"], np.float32)
    te = np.asarray(inputs["tag_embedding"], np.float32)
    ipw = np.asarray(inputs["in_proj_w"], np.float32)
    ipb = np.asarray(inputs["in_proj_b"], np.float32)
    opw = np.asarray(inputs["out_proj_w"], np.float32)
    ob_ = np.asarray(inputs["out_proj_b"], np.float32)
    f1w = np.asarray(inputs["ff1_w"], np.float32)
    f1b = np.asarray(inputs["ff1_b"], np.float32)
    f2w = np.asarray(inputs["ff2_w"], np.float32)
    f2b = np.asarray(inputs["ff2_b"], np.float32)
    lg = np.asarray(inputs["ln_g"], np.float32)
    lb = np.asarray(inputs["ln_b"], np.float32)
    lw = np.asarray(inputs["lin_w"], np.float32)
    lbias = np.asarray(inputs["lin_b"], np.float32)
    sb = np.asarray(inputs["span_batch"]).astype(np.int64)
    st = np.asarray(inputs["span_tag"]).astype(np.int64)
    ss = np.asarray(inputs["span_start"]).astype(np.int64)
    se = np.asarray(inputs["span_end"]).astype(np.int64)

    # ---- parameter-only folds ----------------------------------------
    v_tag = (te @ ipw[2 * H:].T + ipb[2 * H:]) @ opw.T + ob_      # [T, H]
    glw = lg[:, None] * lw.T                                      # [NEW_H, NL]
    c1 = glw.sum(0)                                               # [NL]
    c2 = lw @ lb + lbias                                          # [NL]

    counts_per_b = np.bincount(sb, minlength=B)
    nt = max(1, int(np.ceil(counts_per_b.max() / P)))

    ff2t = _chunked(f2w.T.astype(ml_dtypes.float8_e4m3), KC_H)
    lwg2 = np.ones((P, KC_F, NL + 1), f16)
    lwg2[:, :, 0:NL] = _chunked(glw.astype(f16), KC_F)
    lwcol = np.zeros((P, 75 + 3 * nt), np.float32)
    lwcol[:, 0:6] = f1b.reshape(KC_H, P).T
    lwcol[:, 6:9] = f2b.reshape(KC_H2, P).T
    lwcol[:, 9:42] = c1[None, :]
    lwcol[:, 42:75] = c2[None, :]
    iota_s = np.arange(S, dtype=f16)
    iota_t = np.arange(T, dtype=f16)

    in_maps = []
    for c in range(NCORES):
        # ff1 shard for tags 2c, 2c+1 in [h'-part, jc, (tl,kk), jj] layout
        blk5 = np.empty((P, KC_H, TPC, KC_H, P), np.float32)
        for tl in range(TPC):
            Bm = f1w[:, (TPC * c + tl) * H:(TPC * c + tl + 1) * H]  # [j, h']
            B4 = Bm.reshape(KC_H, P, KC_H, P)          # [jc, jj, kk, hp]
            blk5[:, :, tl, :, :] = B4.transpose(3, 0, 2, 1)
        ff1blk = np.ascontiguousarray(
            blk5.reshape(P, KC_H, TPC * KC_H * P).astype(ml_dtypes.float8_e4m3)
        )

        idx = np.where(sb == c)[0]
        n = len(idx)
        aux = np.zeros((P, A_VT + 12), f16)
        aux[:, A_IS:A_IS + S] = iota_s[None, :]
        aux[:, A_IT:A_IT + T] = iota_t[None, :]
        # v_tag.T cols (kk*2 + tl)
        vt2 = v_tag[TPC * c:TPC * c + TPC].T.astype(f16)   # [H, 2]
        aux[:, A_VT:A_VT + 12] = vt2.reshape(KC_H, P, TPC).transpose(1, 0, 2).reshape(P, 12)
        spcols = np.zeros((3, nt * P), np.float32)
        spcols[0, :n] = ss[idx]
        spcols[1, :n] = se[idx]
        spcols[2, :n] = st[idx]
        lwc = lwcol.copy()
        lwc[:, 75:75 + nt] = spcols[0].reshape(nt, P).T
        lwc[:, 75 + nt:75 + 2 * nt] = spcols[1].reshape(nt, P).T
        lwc[:, 75 + 2 * nt:75 + 3 * nt] = spcols[2].reshape(nt, P).T

        in_maps.append(dict(
            aux=aux,
            ff1blk=ff1blk,
            we_t=_chunked(np.ascontiguousarray(we[c].T).astype(f16), KC_H),
            ff2t=ff2t,
            lwg2=lwg2,
            lwcol=lwc,
        ))

    if nt not in _CACHE:
        _CACHE[nt] = build_kernel(nt)
    nc = _CACHE[nt]

    res = run_bass_kernel_spmd(nc, in_maps, list(range(NCORES)))
    out = np.stack([res.results[c]["out"] for c in range(NCORES)])
    return out.astype(np.float32)


if __name__ == "__main__":
    import reference
    inp = {k: np.asarray(v) for k, v in reference.setup_inputs().items()}
    got = kernel(**inp)
    print("kernel output:", got.shape, got.dtype)


# revision 58
# speedup vs baseline: 1.0481x; 1.0481x over previous
"""Trainium2 Bass kernel for nn_Estor_concat (scatter_memory).

Math (exact reformulation of the reference):
  v_tag = (tag_emb @ Wv.T + bv) @ out_proj_w.T + out_proj_b          [T, H]
  W_eff[t, j] = sum_h v_tag[t, h] * ff1_w[j, t*H + h]               [T, H]
  counts[t, s] = #spans(tag=t, batch=b) covering s   (PE matmul over spans)
  h1 = relu(counts.T @ W_eff + ff1_b); h2 = h1 @ ff2.T + ff2_b
  x = [word_emb_b | h2]; LayerNorm+output folded:
  out = (raw - mu*c1) * rstd + c2,  raw = x @ (g*lin_w.T)

Sharding: data-parallel over batch (8 cores, 1 batch each); W_eff is
sharded over tags (2/core, ff1 rows sharded likewise) and combined with
one AllGather.  v_tag and c1/c2/g·lin_w are parameter-only constants,
folded on the host.  The device schedule dispatches the AllGather as
early as possible (gated only by the fp8 ff1 shard load + ~150ns of N=1
matmuls in transposed j-on-partitions form) and runs counts + the
word-embedding part of the output accumulation inside the collective's
~15.6us latency window, with a dummy-matmul chain keeping the PE pstate
warm.  h2 runs as fp8 DoubleRow matmuls (half row-cost); all LayerNorm
stats are computed in s-on-partitions layout ([128, n] tiles) so the
tail is ~40-160ns ops instead of [1, 512] single-partition ops.
"""

import ml_dtypes
import numpy as np

import concourse.bacc as bacc
import concourse.bass as bass
import concourse.mybir as mybir
import concourse.tile as tile
from concourse.bass_utils import run_bass_kernel_spmd

T, B, S, H = 16, 8, 512, 768
H2 = 384
NEW_H = H + H2          # 1152
NL = 33                 # num labels
EPS = 1e-12
NCORES = 8
TPC = T // NCORES       # tags per core = 2
KC_H = H // 128         # 6
KC_H2 = H2 // 128       # 3
KC_F = NEW_H // 128     # 9
P = 128
NSC = S // 128          # 4 s-chunks
STW = NL + 2            # stats cols per s-chunk: 33 raw + sum + sumsq

F32 = mybir.dt.float32
F16 = mybir.dt.float16
F8 = mybir.dt.float8e4

# aux (fp16) column offsets
A_IS = 0                # iota_s [512]
A_IT = 512              # iota_t [16]
A_VT = 528              # v_tag.T cols (kk*2 + tl) [12]


def build_kernel(nt: int, DUMMY_N: int = 61):
    nc = bacc.Bacc(
        "TRN2",
        target_bir_lowering=False,
        debug=False,
        enable_asserts=True,
        num_devices=NCORES,
    )

    def inp(name, shape, dtype=F32):
        return nc.dram_tensor(name, list(shape), dtype, kind="ExternalInput").ap()

    a_sps = 75
    a_spe = a_sps + nt
    a_spt = a_spe + nt

    aux = inp("aux", (P, A_VT + 12), F16)
    ff1blk = inp("ff1blk", (P, KC_H, TPC * KC_H * P), F8)
    we_t = inp("we_t", (P, KC_H, S), F16)
    ff2t = inp("ff2t", (P, KC_H, H2), F8)
    lwg2 = inp("lwg2", (P, KC_F, NL + 1), F16)
    lwcol = inp("lwcol", (P, 75 + 3 * nt), F32)

    out = nc.dram_tensor("out", [S, NL], F32, kind="ExternalOutput").ap()

    with tile.TileContext(nc) as tc:
        with (
            tc.tile_pool(name="singles", bufs=1) as singles,
            tc.tile_pool(name="spans", bufs=3) as spans,
            tc.tile_pool(name="work", bufs=3) as work,
            tc.tile_pool(name="work2", bufs=2) as work2,
            tc.tile_pool(name="stats", bufs=1) as stats,
            tc.tile_pool(name="fin", bufs=4) as fin,
            tc.tile_pool(name="ps_big", bufs=3, space="PSUM") as ps_big,
            tc.tile_pool(name="ps_h2", bufs=3, space="PSUM") as ps_h2,
            tc.tile_pool(name="ps_acc", bufs=1, space="PSUM") as ps_acc,
            tc.tile_pool(name="dram", bufs=1, space="DRAM") as dram,
        ):
            # ---- constants ------------------------------------------------
            ones_col = singles.tile([P, 1], F16)
            nc.vector.memset(ones_col, 1.0)
            eps_t = singles.tile([1, 1], F32)
            nc.vector.memset(eps_t, EPS)
            eps_col = singles.tile([P, 1], F32)
            nc.vector.memset(eps_col, EPS)
            scratch = singles.tile([1, 1], F32)

            # ---- DMA wave 1: the AllGather path (ff1 shard) gates all -----
            aux_sb = singles.tile([P, A_VT + 12], F16)
            nc.sync.dma_start(out=aux_sb, in_=aux)
            ff1_sb = singles.tile([P, KC_H, TPC * KC_H * P], F8)
            nc.sync.dma_start(out=ff1_sb[:, 0, :], in_=ff1blk[:, 0, :])
            nc.sync.dma_start(out=ff1_sb[:, 1, :], in_=ff1blk[:, 1, :])
            nc.scalar.dma_start(out=ff1_sb[:, 2, :], in_=ff1blk[:, 2, :])
            nc.gpsimd.dma_start(out=ff1_sb[:, 3, :], in_=ff1blk[:, 3, :])
            nc.gpsimd.dma_start(out=ff1_sb[:, 4, :], in_=ff1blk[:, 4, :])
            nc.gpsimd.dma_start(out=ff1_sb[:, 5, :], in_=ff1blk[:, 5, :])
            # ---- wave 2: everything needed during/after the collective ----
            we_sb = singles.tile([P, KC_H, S], F16)
            nc.sync.dma_start(out=we_sb[:, 0:3, :], in_=we_t[:, 0:3, :])
            nc.scalar.dma_start(out=we_sb[:, 3:6, :], in_=we_t[:, 3:6, :])
            ff2_sb = singles.tile([P, KC_H, H2], F8)
            nc.sync.dma_start(out=ff2_sb[:, 0:3, :], in_=ff2t[:, 0:3, :])
            nc.scalar.dma_start(out=ff2_sb[:, 3:6, :], in_=ff2t[:, 3:6, :])
            lwg2_sb = singles.tile([P, KC_F, NL + 1], F16)
            nc.scalar.dma_start(out=lwg2_sb, in_=lwg2)
            lwcol_sb = singles.tile([P, 75 + 3 * nt], F32)
            nc.scalar.dma_start(out=lwcol_sb, in_=lwcol)
            ff1b = lwcol_sb[:, 0:6]
            ff2b = lwcol_sb[:, 6:9]
            c1b = lwcol_sb[:, 9:42]
            c2b = lwcol_sb[:, 42:75]

            # prewarm act table set (relu+sqrt+square live in one set)
            nc.scalar.activation(
                out=scratch, in_=eps_t,
                func=mybir.ActivationFunctionType.Sqrt, bias=eps_t, scale=1.0,
            )

            # ---- W_eff^T: 72 N=1 matmuls, j on partitions -----------------
            # wT cols are tl-major (tl*6 + jc) so the transposed DRAM write
            # below lands as row-major [TPC, H].
            wT_sb = singles.tile([P, TPC * KC_H], F16)
            ps_stats = ps_acc.tile([P, NSC * STW + TPC * KC_H], F32, tag="stats")
            nc.vector.memset(ps_stats, 0.0)
            ps_wall = ps_stats[:, NSC * STW:]
            for jc in (0, 3, 1, 4, 5, 2):
                for tl in range(TPC):
                    col = tl * KC_H + jc
                    for kk in range(KC_H):
                        blk = tl * KC_H + kk
                        nc.tensor.matmul(
                            ps_wall[:, col:col + 1],
                            ff1_sb[:, jc, blk * P:(blk + 1) * P],
                            aux_sb[:, A_VT + kk * TPC + tl:A_VT + kk * TPC + tl + 1],
                            start=False, stop=False, skip_group_check=True,
                        )
            nc.vector.tensor_copy(out=wT_sb, in_=ps_wall)

            # ---- AllGather: [TPC, H] shard -> [T, H] ----------------------
            ag_in = dram.tile([TPC * KC_H, P], F16)
            nc.gpsimd.dma_start(out=ag_in.rearrange("a b -> b a"), in_=wT_sb)
            ag_out = dram.tile([T, H], F16)
            nc.gpsimd.collective_compute(
                "AllGather",
                mybir.AluOpType.bypass,
                replica_groups=[list(range(NCORES))],
                ins=[ag_in.opt()],
                outs=[ag_out.opt()],
            )
            weff_sb = singles.tile([T, H], F16)
            nc.sync.dma_start(out=weff_sb[:, 0:H // 2], in_=ag_out[:, 0:H // 2])
            nc.scalar.dma_start(out=weff_sb[:, H // 2:], in_=ag_out[:, H // 2:])

            # ================ overlapped with the AllGather ================
            # ---- counts: masks on DVE, accumulate on PE -------------------
            counts_ps = ps_acc.tile([T, S], F32, tag="counts")
            tc.tile_set_cur_wait(0.0042)
            for i in range(nt):
                lt_e = spans.tile([P, S], F16, tag="lt_e")
                lt_s = spans.tile([P, S], F16, tag="lt_s")
                mask = spans.tile([P, S], F16, tag="mask")
                nc.vector.tensor_scalar(
                    out=lt_e, in0=aux_sb[:, A_IS:A_IS + S],
                    scalar1=lwcol_sb[:, a_spe + i:a_spe + i + 1], scalar2=None,
                    op0=mybir.AluOpType.is_lt,
                )
                nc.vector.tensor_scalar(
                    out=lt_s, in0=aux_sb[:, A_IS:A_IS + S],
                    scalar1=lwcol_sb[:, a_sps + i:a_sps + i + 1], scalar2=None,
                    op0=mybir.AluOpType.is_ge,
                )
                nc.vector.tensor_mul(out=mask, in0=lt_e, in1=lt_s)
                onehot = spans.tile([P, T], F16, tag="onehot")
                nc.vector.tensor_scalar(
                    out=onehot, in0=aux_sb[:, A_IT:A_IT + T],
                    scalar1=lwcol_sb[:, a_spt + i:a_spt + i + 1], scalar2=None,
                    op0=mybir.AluOpType.is_equal,
                )
                nc.tensor.matmul(
                    counts_ps, onehot, mask,
                    start=(i == 0), stop=(i == nt - 1),
                )
            counts_sb = singles.tile([T, S], F16)
            nc.vector.tensor_copy(out=counts_sb, in_=counts_ps)
            tc.cur_wait_ts = None

            # ---- word-embedding part of raw/sum/sumsq, s-on-partitions ----
            # ps_stats cols per sc: [0:33]=raw, 33=sum, 34=sumsq
            for fc in range(KC_H):
                sq = work.tile([P, S], F16, tag="sq")
                nc.vector.tensor_mul(
                    out=sq, in0=we_sb[:, fc, :], in1=we_sb[:, fc, :]
                )
                for sc in range(NSC):
                    o = sc * STW
                    nc.tensor.matmul(
                        ps_stats[:, o:o + NL + 1],
                        we_sb[:, fc, sc * P:(sc + 1) * P],
                        lwg2_sb[:, fc, :],
                        start=False, stop=False, skip_group_check=True,
                    )
                    nc.tensor.matmul(
                        ps_stats[:, o + NL + 1:o + NL + 2],
                        sq[:, sc * P:(sc + 1) * P],
                        ones_col,
                        start=False, stop=False, skip_group_check=True,
                    )

            # ---- PE keep-warm chain through the collective window ---------
            # WAW-serialized dummy matmuls hold the PE pstate at full clock
            # until W_eff arrives; DUMMY_N is tuned so the chain ends exactly
            # at the weff load (overrun delays h1, undershoot resets pstate).
            for _ in range(DUMMY_N):
                ps_dum = ps_big.tile([TPC * KC_H, S], F32, tag="big")
                nc.tensor.matmul(
                    ps_dum, wT_sb, we_sb[:, 0, :], start=True, stop=True,
                )

            # ================ post-AllGather tail ==========================
            # h1 = relu(counts.T @ W_eff + ff1_b), stored transposed [H, S]
            h1r_sb = singles.tile([P, KC_H, S], F8)
            for kj in range(KC_H):
                # alternate pools so all six chunks get independent buffers
                pool = ps_big if kj % 2 == 0 else ps_h2
                ps = pool.tile([P, S], F32, tag="big" if kj % 2 == 0 else "h2")
                nc.tensor.matmul(
                    ps, weff_sb[:, kj * P:(kj + 1) * P], counts_sb,
                    start=True, stop=True,
                )
                if kj % 2 == 1:
                    nc.scalar.activation(
                        out=h1r_sb[:, kj, :], in_=ps,
                        func=mybir.ActivationFunctionType.Relu,
                        bias=ff1b[:, kj:kj + 1], scale=1.0,
                    )
                else:
                    nc.vector.tensor_scalar(
                        out=h1r_sb[:, kj, :], in0=ps,
                        scalar1=ff1b[:, kj:kj + 1], scalar2=0.0,
                        op0=mybir.AluOpType.add, op1=mybir.AluOpType.max,
                    )

            # h2 = relu_h1 @ ff2.T + ff2_b (fp8 DoubleRow), transposed [H2, S]
            xh2_sb = singles.tile([P, KC_H2, S], F16)
            for mc in range(KC_H2):
                ps = ps_h2.tile([P, S], F32, tag="h2")
                for kjp in range(KC_H // 2):
                    nc.tensor.matmul(
                        ps,
                        ff2_sb[:, 2 * kjp:2 * kjp + 2, mc * P:(mc + 1) * P],
                        h1r_sb[:, 2 * kjp:2 * kjp + 2, :],
                        start=(kjp == 0), stop=(kjp == KC_H // 2 - 1),
                        perf_mode=mybir.MatmulPerfMode.DoubleRow,
                    )
                if mc == 2:
                    # split the critical last chunk across both PSUM engines
                    h = S // 2
                    nc.scalar.activation(
                        out=xh2_sb[:, mc, 0:h], in_=ps[:, 0:h],
                        func=mybir.ActivationFunctionType.Identity,
                        bias=ff2b[:, mc:mc + 1], scale=1.0,
                    )
                    nc.vector.tensor_scalar(
                        out=xh2_sb[:, mc, h:], in0=ps[:, h:],
                        scalar1=ff2b[:, mc:mc + 1], scalar2=None,
                        op0=mybir.AluOpType.add,
                    )
                elif mc == 0:
                    nc.scalar.activation(
                        out=xh2_sb[:, mc, :], in_=ps,
                        func=mybir.ActivationFunctionType.Identity,
                        bias=ff2b[:, mc:mc + 1], scale=1.0,
                    )
                else:
                    nc.vector.tensor_scalar(
                        out=xh2_sb[:, mc, :], in0=ps,
                        scalar1=ff2b[:, mc:mc + 1], scalar2=None,
                        op0=mybir.AluOpType.add,
                    )
                sq = work2.tile([P, S], F16, tag="sqh")
                if mc == 2:
                    h = S // 2
                    nc.vector.tensor_mul(
                        out=sq[:, 0:h], in0=xh2_sb[:, mc, 0:h],
                        in1=xh2_sb[:, mc, 0:h],
                    )
                    nc.gpsimd.tensor_mul(
                        out=sq[:, h:], in0=xh2_sb[:, mc, h:],
                        in1=xh2_sb[:, mc, h:],
                    )
                else:
                    engq = nc.gpsimd if mc == 0 else nc.vector
                    engq.tensor_mul(
                        out=sq, in0=xh2_sb[:, mc, :], in1=xh2_sb[:, mc, :]
                    )
                for sc in range(NSC):
                    o = sc * STW
                    nc.tensor.matmul(
                        ps_stats[:, o:o + NL + 1],
                        xh2_sb[:, mc, sc * P:(sc + 1) * P],
                        lwg2_sb[:, KC_H + mc, :],
                        start=False, stop=False, skip_group_check=True,
                    )
                    nc.tensor.matmul(
                        ps_stats[:, o + NL + 1:o + NL + 2],
                        sq[:, sc * P:(sc + 1) * P],
                        ones_col,
                        start=False, stop=False, skip_group_check=True,
                    )

            # ---- stats + final normalize per s-chunk ----------------------
            mu_t = stats.tile([P, NSC], F32)
            nv_t = stats.tile([P, NSC], F32)
            sd_t = stats.tile([P, NSC], F32)
            rstd_t = stats.tile([P, NSC], F32)
            for sc in range(NSC):
                o = sc * STW
                nc.vector.tensor_scalar_mul(
                    out=mu_t[:, sc:sc + 1], in0=ps_stats[:, o + NL:o + NL + 1],
                    scalar1=1.0 / NEW_H,
                )
                # nv = N*mu^2 - sumsq  ->  var = -nv/N
                nc.vector.tensor_scalar(
                    out=nv_t[:, sc:sc + 1], in0=mu_t[:, sc:sc + 1],
                    scalar1=ps_stats[:, o + NL:o + NL + 1],
                    scalar2=ps_stats[:, o + NL + 1:o + NL + 2],
                    op0=mybir.AluOpType.mult, op1=mybir.AluOpType.subtract,
                )
            # batched sqrt + reciprocal across all four s-chunks
            nc.scalar.activation(
                out=sd_t, in_=nv_t,
                func=mybir.ActivationFunctionType.Sqrt,
                bias=eps_col, scale=-1.0 / NEW_H,
            )
            nc.vector.reciprocal(out=rstd_t, in_=sd_t)
            for sc in range(NSC):
                o = sc * STW
                tmp = fin.tile([P, NL], F32, tag="tmp")
                nc.vector.tensor_scalar_mul(
                    out=tmp, in0=c1b, scalar1=mu_t[:, sc:sc + 1],
                )
                x1 = fin.tile([P, NL], F32, tag="x1")
                nc.vector.tensor_sub(
                    out=x1, in0=ps_stats[:, o:o + NL], in1=tmp
                )
                tt = fin.tile([P, NL], F32, tag="tt")
                nc.gpsimd.tensor_scalar_mul(
                    out=tt, in0=x1, scalar1=rstd_t[:, sc:sc + 1],
                )
                fo = fin.tile([P, NL], F32, tag="fo")
                nc.gpsimd.tensor_add(out=fo, in0=tt, in1=c2b)
                eng = (nc.sync, nc.scalar, nc.sync, nc.scalar)[sc]
                eng.dma_start(out=out[sc * P:(sc + 1) * P, :], in_=fo)

    nc.compile()
    return nc


def _chunked(a, kc):
    """[kc*128, N...] -> [128, kc, N...] (partition-major chunk layout)."""
    return np.ascontiguousarray(
        a.reshape(kc, P, *a.shape[1:]).transpose(1, 0, *range(2, a.ndim + 1))
    )


_CACHE = {}


def kernel(**inputs) -> np.ndarray:
    f16 = np.float16
    we = np.asarray(inputs["word_embedding"], np.float32)
    te = np.asarray(inputs["tag_embedding"], np.float32)
    ipw = np.asarray(inputs["in_proj_w"], np.float32)
    ipb = np.asarray(inputs["in_proj_b"], np.float32)
    opw = np.asarray(inputs["out_proj_w"], np.float32)
    ob_ = np.asarray(inputs["out_proj_b"], np.float32)
    f1w = np.asarray(inputs["ff1_w"], np.float32)
    f1b = np.asarray(inputs["ff1_b"], np.float32)
    f2w = np.asarray(inputs["ff2_w"], np.float32)
    f2b = np.asarray(inputs["ff2_b"], np.float32)
    lg = np.asarray(inputs["ln_g"], np.float32)
    lb = np.asarray(inputs["ln_b"], np.float32)
    lw = np.asarray(inputs["lin_w"], np.float32)
    lbias = np.asarray(inputs["lin_b"], np.float32)
    sb = np.asarray(inputs["span_batch"]).astype(np.int64)
    st = np.asarray(inputs["span_tag"]).astype(np.int64)
    ss = np.asarray(inputs["span_start"]).astype(np.int64)
    se = np.asarray(inputs["span_end"]).astype(np.int64)

    # ---- parameter-only folds ----------------------------------------
    v_tag = (te @ ipw[2 * H:].T + ipb[2 * H:]) @ opw.T + ob_      # [T, H]
    glw = lg[:, None] * lw.T                                      # [NEW_H, NL]
    c1 = glw.sum(0)                                               # [NL]
    c2 = lw @ lb + lbias                                          # [NL]

    counts_per_b = np.bincount(sb, minlength=B)
    nt = max(1, int(np.ceil(counts_per_b.max() / P)))

    ff2t = _chunked(f2w.T.astype(ml_dtypes.float8_e4m3), KC_H)
    lwg2 = np.ones((P, KC_F, NL + 1), f16)
    lwg2[:, :, 0:NL] = _chunked(glw.astype(f16), KC_F)
    lwcol = np.zeros((P, 75 + 3 * nt), np.float32)
    lwcol[:, 0:6] = f1b.reshape(KC_H, P).T
    lwcol[:, 6:9] = f2b.reshape(KC_H2, P).T
    lwcol[:, 9:42] = c1[None, :]
    lwcol[:, 42:75] = c2[None, :]
    iota_s = np.arange(S, dtype=f16)
    iota_t = np.arange(T, dtype=f16)

    in_maps = []
    for c in range(NCORES):
        # ff1 shard for tags 2c, 2c+1 in [h'-part, jc, (tl,kk), jj] layout
        blk5 = np.empty((P, KC_H, TPC, KC_H, P), np.float32)
        for tl in range(TPC):
            Bm = f1w[:, (TPC * c + tl) * H:(TPC * c + tl + 1) * H]  # [j, h']
            B4 = Bm.reshape(KC_H, P, KC_H, P)          # [jc, jj, kk, hp]
            blk5[:, :, tl, :, :] = B4.transpose(3, 0, 2, 1)
        ff1blk = np.ascontiguousarray(
            blk5.reshape(P, KC_H, TPC * KC_H * P).astype(ml_dtypes.float8_e4m3)
        )

        idx = np.where(sb == c)[0]
        n = len(idx)
        aux = np.zeros((P, A_VT + 12), f16)
        aux[:, A_IS:A_IS + S] = iota_s[None, :]
        aux[:, A_IT:A_IT + T] = iota_t[None, :]
        # v_tag.T cols (kk*2 + tl)
        vt2 = v_tag[TPC * c:TPC * c + TPC].T.astype(f16)   # [H, 2]
        aux[:, A_VT:A_VT + 12] = vt2.reshape(KC_H, P, TPC).transpose(1, 0, 2).reshape(P, 12)
        spcols = np.zeros((3, nt * P), np.float32)
        spcols[0, :n] = ss[idx]
        spcols[1, :n] = se[idx]
        spcols[2, :n] = st[idx]
        lwc = lwcol.copy()
        lwc[:, 75:75 + nt] = spcols[0].reshape(nt, P).T
        lwc[:, 75 + nt:75 + 2 * nt] = spcols[1].reshape(nt, P).T
        lwc[:, 75 + 2 * nt:75 + 3 * nt] = spcols[2].reshape(nt, P).T

        in_maps.append(dict(
            aux=aux,
            ff1blk=ff1blk,
            we_t=_chunked(np.ascontiguousarray(we[c].T).astype(f16), KC_H),
            ff2t=ff2t,
            lwg2=lwg2,
            lwcol=lwc,
        ))

    if nt not in _CACHE:
        _CACHE[nt] = build_kernel(nt)
    nc = _CACHE[nt]

    res = run_bass_kernel_spmd(nc, in_maps, list(range(NCORES)))
    out = np.stack([res.results[c]["out"] for c in range(NCORES)])
    return out.astype(np.float32)


if __name__ == "__main__":
    import reference
    inp = {k: np.asarray(v) for k, v in reference.setup_inputs().items()}
    got = kernel(**inp)
    print("kernel output:", got.shape, got.dtype)


# revision 59
# speedup vs baseline: 1.0511x; 1.0029x over previous
"""Trainium2 Bass kernel for nn_Estor_concat (scatter_memory).

Math (exact reformulation of the reference):
  v_tag = (tag_emb @ Wv.T + bv) @ out_proj_w.T + out_proj_b          [T, H]
  W_eff[t, j] = sum_h v_tag[t, h] * ff1_w[j, t*H + h]               [T, H]
  counts[t, s] = #spans(tag=t, batch=b) covering s   (PE matmul over spans)
  h1 = relu(counts.T @ W_eff + ff1_b); h2 = h1 @ ff2.T + ff2_b
  x = [word_emb_b | h2]; LayerNorm+output folded:
  out = (raw - mu*c1) * rstd + c2,  raw = x @ (g*lin_w.T)

Sharding: data-parallel over batch (8 cores, 1 batch each); W_eff is
sharded over tags (2/core, ff1 rows sharded likewise) and combined with
one AllGather.  v_tag and c1/c2/g·lin_w are parameter-only constants,
folded on the host.  The device schedule dispatches the AllGather as
early as possible (gated only by the fp8 ff1 shard load + ~150ns of N=1
matmuls in transposed j-on-partitions form) and runs counts + the
word-embedding part of the output accumulation inside the collective's
~15.6us latency window, with a dummy-matmul chain keeping the PE pstate
warm.  h2 runs as fp8 DoubleRow matmuls (half row-cost); all LayerNorm
stats are computed in s-on-partitions layout ([128, n] tiles) so the
tail is ~40-160ns ops instead of [1, 512] single-partition ops.
"""

import ml_dtypes
import numpy as np

import concourse.bacc as bacc
import concourse.bass as bass
import concourse.mybir as mybir
import concourse.tile as tile
from concourse.bass_utils import run_bass_kernel_spmd

T, B, S, H = 16, 8, 512, 768
H2 = 384
NEW_H = H + H2          # 1152
NL = 33                 # num labels
EPS = 1e-12
NCORES = 8
TPC = T // NCORES       # tags per core = 2
KC_H = H // 128         # 6
KC_H2 = H2 // 128       # 3
KC_F = NEW_H // 128     # 9
P = 128
NSC = S // 128          # 4 s-chunks
STW = NL + 2            # stats cols per s-chunk: 33 raw + sum + sumsq

F32 = mybir.dt.float32
F16 = mybir.dt.float16
F8 = mybir.dt.float8e4

# aux (fp16) column offsets
A_IS = 0                # iota_s [512]
A_IT = 512              # iota_t [16]
A_VT = 528              # v_tag.T cols (kk*2 + tl) [12]


def build_kernel(nt: int, DUMMY_N: int = 61):
    nc = bacc.Bacc(
        "TRN2",
        target_bir_lowering=False,
        debug=False,
        enable_asserts=True,
        num_devices=NCORES,
    )

    def inp(name, shape, dtype=F32):
        return nc.dram_tensor(name, list(shape), dtype, kind="ExternalInput").ap()

    a_sps = 75
    a_spe = a_sps + nt
    a_spt = a_spe + nt

    aux = inp("aux", (P, A_VT + 12), F16)
    ff1blk = inp("ff1blk", (P, KC_H, TPC * KC_H * P), F8)
    we_t = inp("we_t", (P, KC_H, S), F16)
    ff2t = inp("ff2t", (P, KC_H, H2), F8)
    lwg2 = inp("lwg2", (P, KC_F, NL + 1), F16)
    lwcol = inp("lwcol", (P, 75 + 3 * nt), F32)

    out = nc.dram_tensor("out", [S, NL], F32, kind="ExternalOutput").ap()

    with tile.TileContext(nc) as tc:
        with (
            tc.tile_pool(name="singles", bufs=1) as singles,
            tc.tile_pool(name="spans", bufs=3) as spans,
            tc.tile_pool(name="work", bufs=3) as work,
            tc.tile_pool(name="work2", bufs=2) as work2,
            tc.tile_pool(name="stats", bufs=1) as stats,
            tc.tile_pool(name="fin", bufs=4) as fin,
            tc.tile_pool(name="ps_big", bufs=3, space="PSUM") as ps_big,
            tc.tile_pool(name="ps_h2", bufs=3, space="PSUM") as ps_h2,
            tc.tile_pool(name="ps_acc", bufs=1, space="PSUM") as ps_acc,
            tc.tile_pool(name="dram", bufs=1, space="DRAM") as dram,
        ):
            # ---- constants ------------------------------------------------
            ones_col = singles.tile([P, 1], F16)
            nc.vector.memset(ones_col, 1.0)
            eps_t = singles.tile([1, 1], F32)
            nc.vector.memset(eps_t, EPS)
            eps_col = singles.tile([P, 1], F32)
            nc.vector.memset(eps_col, EPS)
            scratch = singles.tile([1, 1], F32)

            # ---- DMA wave 1: the AllGather path (ff1 shard) gates all -----
            aux_sb = singles.tile([P, A_VT + 12], F16)
            nc.sync.dma_start(out=aux_sb, in_=aux)
            ff1_sb = singles.tile([P, KC_H, TPC * KC_H * P], F8)
            nc.sync.dma_start(out=ff1_sb[:, 0, :], in_=ff1blk[:, 0, :])
            nc.sync.dma_start(out=ff1_sb[:, 1, :], in_=ff1blk[:, 1, :])
            nc.scalar.dma_start(out=ff1_sb[:, 2, :], in_=ff1blk[:, 2, :])
            nc.gpsimd.dma_start(out=ff1_sb[:, 3, :], in_=ff1blk[:, 3, :])
            nc.gpsimd.dma_start(out=ff1_sb[:, 4, :], in_=ff1blk[:, 4, :])
            nc.gpsimd.dma_start(out=ff1_sb[:, 5, :], in_=ff1blk[:, 5, :])
            # ---- wave 2: everything needed during/after the collective ----
            we_sb = singles.tile([P, KC_H, S], F16)
            nc.sync.dma_start(out=we_sb[:, 0:3, :], in_=we_t[:, 0:3, :])
            nc.scalar.dma_start(out=we_sb[:, 3:6, :], in_=we_t[:, 3:6, :])
            ff2_sb = singles.tile([P, KC_H, H2], F8)
            nc.sync.dma_start(out=ff2_sb[:, 0:3, :], in_=ff2t[:, 0:3, :])
            nc.scalar.dma_start(out=ff2_sb[:, 3:6, :], in_=ff2t[:, 3:6, :])
            lwg2_sb = singles.tile([P, KC_F, NL + 1], F16)
            nc.scalar.dma_start(out=lwg2_sb, in_=lwg2)
            lwcol_sb = singles.tile([P, 75 + 3 * nt], F32)
            nc.scalar.dma_start(out=lwcol_sb, in_=lwcol)
            ff1b = lwcol_sb[:, 0:6]
            ff2b = lwcol_sb[:, 6:9]
            c1b = lwcol_sb[:, 9:42]
            c2b = lwcol_sb[:, 42:75]

            # prewarm act table set (relu+sqrt+square live in one set)
            nc.scalar.activation(
                out=scratch, in_=eps_t,
                func=mybir.ActivationFunctionType.Sqrt, bias=eps_t, scale=1.0,
            )

            # ---- W_eff^T: 72 N=1 matmuls, j on partitions -----------------
            # wT cols are tl-major (tl*6 + jc) so the transposed DRAM write
            # below lands as row-major [TPC, H].
            wT_sb = singles.tile([P, TPC * KC_H], F16)
            ps_stats = ps_acc.tile([P, NSC * STW + TPC * KC_H], F32, tag="stats")
            nc.vector.memset(ps_stats, 0.0)
            ps_wall = ps_stats[:, NSC * STW:]
            for jc in (0, 3, 1, 4, 5, 2):
                for tl in range(TPC):
                    col = tl * KC_H + jc
                    for kk in range(KC_H):
                        blk = tl * KC_H + kk
                        nc.tensor.matmul(
                            ps_wall[:, col:col + 1],
                            ff1_sb[:, jc, blk * P:(blk + 1) * P],
                            aux_sb[:, A_VT + kk * TPC + tl:A_VT + kk * TPC + tl + 1],
                            start=False, stop=False, skip_group_check=True,
                        )
            nc.vector.tensor_copy(out=wT_sb, in_=ps_wall)

            # ---- AllGather: [TPC, H] shard -> [T, H] ----------------------
            ag_in = dram.tile([TPC * KC_H, P], F16)
            nc.gpsimd.dma_start(out=ag_in.rearrange("a b -> b a"), in_=wT_sb)
            ag_out = dram.tile([T, H], F16)
            nc.gpsimd.collective_compute(
                "AllGather",
                mybir.AluOpType.bypass,
                replica_groups=[list(range(NCORES))],
                ins=[ag_in.opt()],
                outs=[ag_out.opt()],
            )
            weff_sb = singles.tile([T, H], F16)
            nc.sync.dma_start(out=weff_sb[:, 0:H // 2], in_=ag_out[:, 0:H // 2])
            nc.scalar.dma_start(out=weff_sb[:, H // 2:], in_=ag_out[:, H // 2:])

            # ================ overlapped with the AllGather ================
            # ---- counts: masks on DVE, accumulate on PE -------------------
            counts_ps = ps_acc.tile([T, S], F32, tag="counts")
            tc.tile_set_cur_wait(0.0042)
            for i in range(nt):
                lt_e = spans.tile([P, S], F16, tag="lt_e")
                lt_s = spans.tile([P, S], F16, tag="lt_s")
                mask = spans.tile([P, S], F16, tag="mask")
                nc.vector.tensor_scalar(
                    out=lt_e, in0=aux_sb[:, A_IS:A_IS + S],
                    scalar1=lwcol_sb[:, a_spe + i:a_spe + i + 1], scalar2=None,
                    op0=mybir.AluOpType.is_lt,
                )
                nc.vector.tensor_scalar(
                    out=lt_s, in0=aux_sb[:, A_IS:A_IS + S],
                    scalar1=lwcol_sb[:, a_sps + i:a_sps + i + 1], scalar2=None,
                    op0=mybir.AluOpType.is_ge,
                )
                nc.vector.tensor_mul(out=mask, in0=lt_e, in1=lt_s)
                onehot = spans.tile([P, T], F16, tag="onehot")
                nc.vector.tensor_scalar(
                    out=onehot, in0=aux_sb[:, A_IT:A_IT + T],
                    scalar1=lwcol_sb[:, a_spt + i:a_spt + i + 1], scalar2=None,
                    op0=mybir.AluOpType.is_equal,
                )
                nc.tensor.matmul(
                    counts_ps, onehot, mask,
                    start=(i == 0), stop=(i == nt - 1),
                )
            counts_sb = singles.tile([T, S], F16)
            nc.vector.tensor_copy(out=counts_sb, in_=counts_ps)
            tc.cur_wait_ts = None

            # ---- word-embedding part of raw/sum/sumsq, s-on-partitions ----
            # ps_stats cols per sc: [0:33]=raw, 33=sum, 34=sumsq
            for fc in range(KC_H):
                sq = work.tile([P, S], F16, tag="sq")
                nc.vector.tensor_mul(
                    out=sq, in0=we_sb[:, fc, :], in1=we_sb[:, fc, :]
                )
                for sc in range(NSC):
                    o = sc * STW
                    nc.tensor.matmul(
                        ps_stats[:, o:o + NL + 1],
                        we_sb[:, fc, sc * P:(sc + 1) * P],
                        lwg2_sb[:, fc, :],
                        start=False, stop=False, skip_group_check=True,
                    )
                    nc.tensor.matmul(
                        ps_stats[:, o + NL + 1:o + NL + 2],
                        sq[:, sc * P:(sc + 1) * P],
                        ones_col,
                        start=False, stop=False, skip_group_check=True,
                    )

            # ---- PE keep-warm chain through the collective window ---------
            # WAW-serialized dummy matmuls hold the PE pstate at full clock
            # until W_eff arrives; DUMMY_N is tuned so the chain ends exactly
            # at the weff load (overrun delays h1, undershoot resets pstate).
            for _ in range(DUMMY_N):
                ps_dum = ps_big.tile([TPC * KC_H, S], F32, tag="big")
                nc.tensor.matmul(
                    ps_dum, wT_sb, we_sb[:, 0, :], start=True, stop=True,
                )

            # ================ post-AllGather tail ==========================
            # h1 = relu(counts.T @ W_eff + ff1_b), stored transposed [H, S]
            h1r_sb = singles.tile([P, KC_H, S], F8)
            for kj in range(KC_H):
                # alternate pools so all six chunks get independent buffers
                pool = ps_big if kj % 2 == 0 else ps_h2
                ps = pool.tile([P, S], F32, tag="big" if kj % 2 == 0 else "h2")
                nc.tensor.matmul(
                    ps, weff_sb[:, kj * P:(kj + 1) * P], counts_sb,
                    start=True, stop=True,
                )
                if kj % 2 == 1:
                    nc.scalar.activation(
                        out=h1r_sb[:, kj, :], in_=ps,
                        func=mybir.ActivationFunctionType.Relu,
                        bias=ff1b[:, kj:kj + 1], scale=1.0,
                    )
                else:
                    nc.vector.tensor_scalar(
                        out=h1r_sb[:, kj, :], in0=ps,
                        scalar1=ff1b[:, kj:kj + 1], scalar2=0.0,
                        op0=mybir.AluOpType.add, op1=mybir.AluOpType.max,
                    )

            # h2 = relu_h1 @ ff2.T + ff2_b (fp8 DoubleRow), transposed [H2, S]
            xh2_sb = singles.tile([P, KC_H2, S], F16)
            for mc in range(KC_H2):
                ps = ps_h2.tile([P, S], F32, tag="h2")
                for kjp in range(KC_H // 2):
                    nc.tensor.matmul(
                        ps,
                        ff2_sb[:, 2 * kjp:2 * kjp + 2, mc * P:(mc + 1) * P],
                        h1r_sb[:, 2 * kjp:2 * kjp + 2, :],
                        start=(kjp == 0), stop=(kjp == KC_H // 2 - 1),
                        perf_mode=mybir.MatmulPerfMode.DoubleRow,
                    )
                if mc == 2:
                    # split the critical last chunk across both PSUM engines
                    h = S // 2
                    nc.scalar.activation(
                        out=xh2_sb[:, mc, 0:h], in_=ps[:, 0:h],
                        func=mybir.ActivationFunctionType.Identity,
                        bias=ff2b[:, mc:mc + 1], scale=1.0,
                    )
                    nc.vector.tensor_scalar(
                        out=xh2_sb[:, mc, h:], in0=ps[:, h:],
                        scalar1=ff2b[:, mc:mc + 1], scalar2=None,
                        op0=mybir.AluOpType.add,
                    )
                elif mc == 0:
                    nc.scalar.activation(
                        out=xh2_sb[:, mc, :], in_=ps,
                        func=mybir.ActivationFunctionType.Identity,
                        bias=ff2b[:, mc:mc + 1], scale=1.0,
                    )
                else:
                    nc.vector.tensor_scalar(
                        out=xh2_sb[:, mc, :], in0=ps,
                        scalar1=ff2b[:, mc:mc + 1], scalar2=None,
                        op0=mybir.AluOpType.add,
                    )
                sq = work2.tile([P, S], F16, tag="sqh")
                if mc == 2:
                    h = S // 2
                    nc.vector.tensor_mul(
                        out=sq[:, 0:h], in0=xh2_sb[:, mc, 0:h],
                        in1=xh2_sb[:, mc, 0:h],
                    )
                    nc.gpsimd.tensor_mul(
                        out=sq[:, h:], in0=xh2_sb[:, mc, h:],
                        in1=xh2_sb[:, mc, h:],
                    )
                else:
                    engq = nc.gpsimd if mc == 0 else nc.vector
                    engq.tensor_mul(
                        out=sq, in0=xh2_sb[:, mc, :], in1=xh2_sb[:, mc, :]
                    )
                for sc in range(NSC):
                    o = sc * STW
                    nc.tensor.matmul(
                        ps_stats[:, o:o + NL + 1],
                        xh2_sb[:, mc, sc * P:(sc + 1) * P],
                        lwg2_sb[:, KC_H + mc, :],
                        start=False, stop=False, skip_group_check=True,
                    )
                    nc.tensor.matmul(
                        ps_stats[:, o + NL + 1:o + NL + 2],
                        sq[:, sc * P:(sc + 1) * P],
                        ones_col,
                        start=False, stop=False, skip_group_check=True,
                    )

            # ---- stats + final normalize per s-chunk ----------------------
            mu_t = stats.tile([P, NSC], F32)
            nv_t = stats.tile([P, NSC], F32)
            sd_t = stats.tile([P, NSC], F32)
            rstd_t = stats.tile([P, NSC], F32)
            for sc in range(NSC):
                o = sc * STW
                nc.vector.tensor_scalar_mul(
                    out=mu_t[:, sc:sc + 1], in0=ps_stats[:, o + NL:o + NL + 1],
                    scalar1=1.0 / NEW_H,
                )
                # nv = N*mu^2 - sumsq  ->  var = -nv/N
                nc.vector.tensor_scalar(
                    out=nv_t[:, sc:sc + 1], in0=mu_t[:, sc:sc + 1],
                    scalar1=ps_stats[:, o + NL:o + NL + 1],
                    scalar2=ps_stats[:, o + NL + 1:o + NL + 2],
                    op0=mybir.AluOpType.mult, op1=mybir.AluOpType.subtract,
                )
            # batched sqrt + reciprocal across all four s-chunks
            nc.scalar.activation(
                out=sd_t, in_=nv_t,
                func=mybir.ActivationFunctionType.Sqrt,
                bias=eps_col, scale=-1.0 / NEW_H,
            )
            nc.vector.reciprocal(out=rstd_t, in_=sd_t)
            for sc in range(NSC):
                o = sc * STW
                tmp = fin.tile([P, NL], F32, tag="tmp")
                nc.gpsimd.tensor_scalar_mul(
                    out=tmp, in0=c1b, scalar1=mu_t[:, sc:sc + 1],
                )
                x1 = fin.tile([P, NL], F32, tag="x1")
                nc.vector.tensor_sub(
                    out=x1, in0=ps_stats[:, o:o + NL], in1=tmp
                )
                tt = fin.tile([P, NL], F32, tag="tt")
                nc.gpsimd.tensor_scalar_mul(
                    out=tt, in0=x1, scalar1=rstd_t[:, sc:sc + 1],
                )
                fo = fin.tile([P, NL], F32, tag="fo")
                nc.gpsimd.tensor_add(out=fo, in0=tt, in1=c2b)
                eng = (nc.sync, nc.scalar, nc.sync, nc.scalar)[sc]
                eng.dma_start(out=out[sc * P:(sc + 1) * P, :], in_=fo)

    nc.compile()
    return nc


def _chunked(a, kc):
    """[kc*128, N...] -> [128, kc, N...] (partition-major chunk layout)."""
    return np.ascontiguousarray(
        a.reshape(kc, P, *a.shape[1:]).transpose(1, 0, *range(2, a.ndim + 1))
    )


_CACHE = {}


def kernel(**inputs) -> np.ndarray:
    f16 = np.float16
    we = np.asarray(inputs["word_embedding"], np.float32)
    te = np.asarray(inputs["tag_embedding"], np.float32)
    ipw = np.asarray(inputs["in_proj_w"], np.float32)
    ipb = np.asarray(inputs["in_proj_b"], np.float32)
    opw = np.asarray(inputs["out_proj_w"], np.float32)
    ob_ = np.asarray(inputs["out_proj_b"], np.float32)
    f1w = np.asarray(inputs["ff1_w"], np.float32)
    f1b = np.asarray(inputs["ff1_b"], np.float32)
    f2w = np.asarray(inputs["ff2_w"], np.float32)
    f2b = np.asarray(inputs["ff2_b"], np.float32)
    lg = np.asarray(inputs["ln_g"], np.float32)
    lb = np.asarray(inputs["ln_b"], np.float32)
    lw = np.asarray(inputs["lin_w"], np.float32)
    lbias = np.asarray(inputs["lin_b"], np.float32)
    sb = np.asarray(inputs["span_batch"]).astype(np.int64)
    st = np.asarray(inputs["span_tag"]).astype(np.int64)
    ss = np.asarray(inputs["span_start"]).astype(np.int64)
    se = np.asarray(inputs["span_end"]).astype(np.int64)

    # ---- parameter-only folds ----------------------------------------
    v_tag = (te @ ipw[2 * H:].T + ipb[2 * H:]) @ opw.T + ob_      # [T, H]
    glw = lg[:, None] * lw.T                                      # [NEW_H, NL]
    c1 = glw.sum(0)                                               # [NL]
    c2 = lw @ lb + lbias                                          # [NL]

    counts_per_b = np.bincount(sb, minlength=B)
    nt = max(1, int(np.ceil(counts_per_b.max() / P)))

    ff2t = _chunked(f2w.T.astype(ml_dtypes.float8_e4m3), KC_H)
    lwg2 = np.ones((P, KC_F, NL + 1), f16)
    lwg2[:, :, 0:NL] = _chunked(glw.astype(f16), KC_F)
    lwcol = np.zeros((P, 75 + 3 * nt), np.float32)
    lwcol[:, 0:6] = f1b.reshape(KC_H, P).T
    lwcol[:, 6:9] = f2b.reshape(KC_H2, P).T
    lwcol[:, 9:42] = c1[None, :]
    lwcol[:, 42:75] = c2[None, :]
    iota_s = np.arange(S, dtype=f16)
    iota_t = np.arange(T, dtype=f16)

    in_maps = []
    for c in range(NCORES):
        # ff1 shard for tags 2c, 2c+1 in [h'-part, jc, (tl,kk), jj] layout
        blk5 = np.empty((P, KC_H, TPC, KC_H, P), np.float32)
        for tl in range(TPC):
            Bm = f1w[:, (TPC * c + tl) * H:(TPC * c + tl + 1) * H]  # [j, h']
            B4 = Bm.reshape(KC_H, P, KC_H, P)          # [jc, jj, kk, hp]
            blk5[:, :, tl, :, :] = B4.transpose(3, 0, 2, 1)
        ff1blk = np.ascontiguousarray(
            blk5.reshape(P, KC_H, TPC * KC_H * P).astype(ml_dtypes.float8_e4m3)
        )

        idx = np.where(sb == c)[0]
        n = len(idx)
        aux = np.zeros((P, A_VT + 12), f16)
        aux[:, A_IS:A_IS + S] = iota_s[None, :]
        aux[:, A_IT:A_IT + T] = iota_t[None, :]
        # v_tag.T cols (kk*2 + tl)
        vt2 = v_tag[TPC * c:TPC * c + TPC].T.astype(f16)   # [H, 2]
        aux[:, A_VT:A_VT + 12] = vt2.reshape(KC_H, P, TPC).transpose(1, 0, 2).reshape(P, 12)
        spcols = np.zeros((3, nt * P), np.float32)
        spcols[0, :n] = ss[idx]
        spcols[1, :n] = se[idx]
        spcols[2, :n] = st[idx]
        lwc = lwcol.copy()
        lwc[:, 75:75 + nt] = spcols[0].reshape(nt, P).T
        lwc[:, 75 + nt:75 + 2 * nt] = spcols[1].reshape(nt, P).T
        lwc[:, 75 + 2 * nt:75 + 3 * nt] = spcols[2].reshape(nt, P).T

        in_maps.append(dict(
            aux=aux,
            ff1blk=ff1blk,
            we_t=_chunked(np.ascontiguousarray(we[c].T).astype(f16), KC_H),
            ff2t=ff2t,
            lwg2=lwg2,
            lwcol=lwc,
        ))

    if nt not in _CACHE:
        _CACHE[nt] = build_kernel(nt)
    nc = _CACHE[nt]

    res = run_bass_kernel_spmd(nc, in_maps, list(range(NCORES)))
    out = np.stack([res.results[c]["out"] for c in range(NCORES)])
    return out.astype(np.float32)


if __name__ == "__main__":
    import reference
    inp = {k: np.asarray(v) for k, v in reference.setup_inputs().items()}
    got = kernel(**inp)
    print("kernel output:", got.shape, got.dtype)
